# revision 27
# baseline (speedup 1.0000x reference)
"""Trainium2 Bass kernel for nn_GapDecoder.

Computes gaps[i,j] = proj[i] + proj[j] + b2 where
proj = relu(x @ W1 + b1) @ w2, x: [8192, 512] f32.

Strategy (8 NeuronCores, block-partitioned, collective-free):
  The [8192, 8192] output is an 8x8 grid of [1024, 1024] blocks. Core m
  handles chunk set Lm = {m, m+1, m+2, m+4} (mod 8) and emits the 8
  blocks given by the uniform local pattern
      {(0,0),(0,1),(0,2),(0,3),(1,3),(1,0),(3,1),(3,2)}
  over Lm. One cell per difference delta = Lm[q]-Lm[p] (mod 8) makes the
  union over cores an exact partition of all 64 blocks. Each core reads
  just its 4 x-chunks (8MB, transposed on host so the PE contracts over
  D directly), computes proj for those 4096 rows, broadcasts the
  column-direction proj across partitions with rank-1 PE matmuls, and
  writes each block as 8 chunks of [128, 1024]: DVE tensor_scalar add of
  the per-partition row proj, then a DMA store. 40MB of HBM traffic per
  core (vs 48MB row-sharded) and no cross-core dependency, so staggered
  core starts don't serialize anything.
"""

import sys

sys.path.insert(0, "/opt/trn_rl_repo")

import numpy as np

N, D, H = 8192, 512, 32
NCORES = 8
CHUNK = 1024  # block edge / proj chunk
NLOC = 4  # chunks per core
LROWS = NLOC * CHUNK  # local rows per core
STRIPE = 512  # rows per PE stripe
NSTRIP = LROWS // STRIPE
KCH = D // 128

# local chunk offsets and the block pattern (see module docstring)
LOCAL_OFFS = (0, 1, 2, 4)
PATTERN = ((0, 0), (0, 1), (0, 2), (0, 3), (1, 3), (1, 0), (3, 1), (3, 2))

_state = {}

# Set by run for test harnesses that want profile info (see test.py).
LAST_RESULTS = None


def _build():
    from concourse import bacc, tile, mybir

    f32 = mybir.dt.float32
    nc = bacc.Bacc(
        "TRN2", target_bir_lowering=False, debug=False, num_devices=NCORES
    )

    xT_d = nc.dram_tensor("xT4", [D, LROWS], f32, kind="ExternalInput")
    w1_d = nc.dram_tensor("W1", [D, H], f32, kind="ExternalInput")
    b1_d = nc.dram_tensor("b1c", [H, 1], f32, kind="ExternalInput")
    w2_d = nc.dram_tensor("w2c", [H, 1], f32, kind="ExternalInput")
    # w2 replicated across 128 columns: matmul(lhsT=w2b, rhs=seqT) puts
    # proj[f] on every partition in one step (the column broadcast)
    w2b_d = nc.dram_tensor("w2b", [H, 128], f32, kind="ExternalInput")
    b2b_d = nc.dram_tensor("b2b", [128, 1], f32, kind="ExternalInput")
    # 8 blocks of [CHUNK, CHUNK], stacked along rows
    out_d = nc.dram_tensor("out", [8 * CHUNK, CHUNK], f32, kind="ExternalOutput")

    with tile.TileContext(nc) as tc:
        with (
            tc.tile_pool(name="const", bufs=1) as cpool,
            tc.tile_pool(name="xkp", bufs=4) as xkpool,
            tc.tile_pool(name="work", bufs=2) as wpool,
            tc.tile_pool(name="big", bufs=10) as bigpool,
            tc.tile_pool(name="psum", bufs=2, space="PSUM") as pspool,
            tc.tile_pool(name="psbc", bufs=2, space="PSUM") as psbc,
        ):
            # ---- constants ----
            w1_sb = cpool.tile([128, KCH, H], f32)
            nc.sync.dma_start(
                w1_sb[:], w1_d.ap().rearrange("(k p) h -> p k h", p=128)
            )
            b1_sb = cpool.tile([H, 1], f32)
            nc.sync.dma_start(b1_sb[:], b1_d.ap())
            w2_sb = cpool.tile([H, 1], f32)
            nc.sync.dma_start(w2_sb[:], w2_d.ap())
            w2b_sb = cpool.tile([H, 128], f32)
            nc.sync.dma_start(w2b_sb[:], w2b_d.ap())
            b2b_sb = cpool.tile([128, 1], f32)
            nc.sync.dma_start(b2b_sb[:], b2b_d.ap())

            # per-partition proj scalars ([128, CHUNK//128] per local chunk)
            projcol = [
                cpool.tile([128, CHUNK // 128], f32, name=f"projcol{i}")
                for i in range(NLOC)
            ]
            bcol = [
                cpool.tile([128, CHUNK], f32, name=f"bcol{i}") for i in range(NLOC)
            ]

            # ---- per chunk: proj stripes, then its broadcast, then every
            # block that just became ready — so output DMAs start as soon
            # as the first chunk's proj exists and overlap later compute.
            COMPUTE_ORDER = (0, 1, 3, 2)
            ready = {loc: i for i, loc in enumerate(COMPUTE_ORDER)}
            emitted = set()

            def emit_block(k):
                # alternate the adds between DVE and ACT so neither queue
                # backs up behind the other chunk-compute work
                p, q = PATTERN[k]
                for g in range(CHUNK // 128):
                    ot = bigpool.tile([128, CHUNK], f32, tag="ot", name="ot")
                    # all block adds on ACT: DVE stays exclusively on the
                    # chunk-compute path so the PE never waits on it
                    nc.scalar.add(ot[:], bcol[q][:], projcol[p][:, g : g + 1])
                    r0 = k * CHUNK + g * 128
                    nc.sync.dma_start(out_d.ap()[r0 : r0 + 128, :], ot[:])

            for loc in COMPUTE_ORDER:
              # one whole-chunk load: 4KB-per-partition descriptor runs
              xk = xkpool.tile([128, KCH, CHUNK], f32, tag="xk")
              nc.sync.dma_start(
                  xk[:],
                  xT_d.ap()[:, loc * CHUNK : (loc + 1) * CHUNK].rearrange(
                      "(k p) j -> p k j", p=128
                  ),
              )
              for half in range(CHUNK // STRIPE):
                seqT_ps = pspool.tile([H, STRIPE], f32, tag="seqT")
                for k in range(KCH):
                    nc.tensor.matmul(
                        seqT_ps[:],
                        w1_sb[:, k, :],
                        xk[:, k, half * STRIPE : (half + 1) * STRIPE],
                        start=(k == 0),
                        stop=(k == KCH - 1),
                    )
                seqT_sb = wpool.tile([H, STRIPE], f32, tag="seqT_sb")
                # relu(x + b1) as a fused DVE op, keeping ACT free for the
                # block adds (and avoiding activation-table switching)
                nc.vector.tensor_scalar(
                    seqT_sb[:],
                    seqT_ps[:],
                    b1_sb[:],
                    0.0,
                    op0=mybir.AluOpType.add,
                    op1=mybir.AluOpType.max,
                )
                # broadcast proj of this stripe across all 128 partitions in
                # one matmul, folding b2 into the psum->sbuf copy
                bc_ps = psbc.tile([128, STRIPE], f32, tag="bc")
                nc.tensor.matmul(bc_ps[:], w2b_sb[:], seqT_sb[:])
                nc.vector.tensor_scalar_add(
                    bcol[loc][:, half * STRIPE : (half + 1) * STRIPE],
                    bc_ps[:],
                    b2b_sb[:],
                )
                # local 2 never appears as a block row; skip its scalars
                for c in range(STRIPE // 128) if loc != 2 else ():
                    pc_ps = pspool.tile([128, 1], f32, tag="pc")
                    nc.tensor.matmul(
                        pc_ps[:],
                        seqT_sb[:, c * 128 : (c + 1) * 128],
                        w2_sb[:],
                    )
                    col = half * (STRIPE // 128) + c
                    nc.vector.tensor_copy(projcol[loc][:, col : col + 1], pc_ps[:])

              # emit blocks one stage behind compute: everything ready as of
              # the PREVIOUS chunk. This keeps the next chunk's PSUM-freeing
              # DVE ops ahead of the block adds in each engine's queue, so
              # the PE never stalls on block-emission progress.
              stage = ready[loc]
              for k in range(8):
                  p, q = PATTERN[k]
                  if k not in emitted and max(ready[p], ready[q]) <= max(
                      stage - 1, 0 if stage == 0 else -1
                  ):
                      emitted.add(k)
                      emit_block(k)

            # drain whatever is left (the last two stages)
            for k in range(8):
                if k not in emitted:
                    emitted.add(k)
                    emit_block(k)

    nc.compile()
    return nc


def kernel(gathered_sequences, W1, b1, w2, b2):
    global LAST_RESULTS
    from concourse import bass_utils

    if "nc" not in _state:
        _state["nc"] = _build()
    nc = _state["nc"]

    x = np.ascontiguousarray(gathered_sequences, dtype=np.float32)
    xT = np.ascontiguousarray(x.T)  # [D, N]
    W1 = np.ascontiguousarray(W1, dtype=np.float32)
    b1c = np.ascontiguousarray(np.reshape(b1, (H, 1)), dtype=np.float32)
    w2c = np.ascontiguousarray(np.reshape(w2, (H, 1)), dtype=np.float32)
    w2b = np.ascontiguousarray(np.repeat(w2c, 128, axis=1))
    b2b = np.full((128, 1), np.reshape(b2, ()), dtype=np.float32)

    in_maps = []
    for m in range(NCORES):
        locs = [(m + a) % NCORES for a in LOCAL_OFFS]
        xT4 = np.concatenate(
            [xT[:, L * CHUNK : (L + 1) * CHUNK] for L in locs], axis=1
        )
        in_maps.append(
            {
                "xT4": np.ascontiguousarray(xT4),
                "W1": W1,
                "b1c": b1c,
                "w2c": w2c,
                "w2b": w2b,
                "b2b": b2b,
            }
        )

    res = bass_utils.run_bass_kernel_spmd(nc, in_maps, core_ids=list(range(NCORES)))
    LAST_RESULTS = res

    out = np.empty((N, N), dtype=np.float32)
    for m in range(NCORES):
        locs = [(m + a) % NCORES for a in LOCAL_OFFS]
        blocks = res.results[m]["out"]
        for k, (p, q) in enumerate(PATTERN):
            gr, gc = locs[p], locs[q]
            out[gr * CHUNK : (gr + 1) * CHUNK, gc * CHUNK : (gc + 1) * CHUNK] = (
                blocks[k * CHUNK : (k + 1) * CHUNK, :]
            )
    return out


# revision 29
# speedup vs baseline: 1.0236x; 1.0236x over previous
"""Trainium2 Bass kernel for nn_GapDecoder.

Computes gaps[i,j] = proj[i] + proj[j] + b2 where
proj = relu(x @ W1 + b1) @ w2, x: [8192, 512] f32.

Strategy (8 NeuronCores, block-partitioned, collective-free):
  The [8192, 8192] output is an 8x8 grid of [1024, 1024] blocks. Core m
  handles chunk set Lm = {m, m+1, m+2, m+4} (mod 8) and emits the 8
  blocks given by the uniform local pattern
      {(0,0),(0,1),(0,2),(0,3),(1,3),(1,0),(3,1),(3,2)}
  over Lm. One cell per difference delta = Lm[q]-Lm[p] (mod 8) makes the
  union over cores an exact partition of all 64 blocks. Each core reads
  just its 4 x-chunks (8MB, transposed on host so the PE contracts over
  D directly), computes proj for those 4096 rows, broadcasts the
  column-direction proj across partitions with rank-1 PE matmuls, and
  writes each block as 8 chunks of [128, 1024]: DVE tensor_scalar add of
  the per-partition row proj, then a DMA store. 40MB of HBM traffic per
  core (vs 48MB row-sharded) and no cross-core dependency, so staggered
  core starts don't serialize anything.
"""

import sys

sys.path.insert(0, "/opt/trn_rl_repo")

import numpy as np

N, D, H = 8192, 512, 32
NCORES = 8
CHUNK = 1024  # block edge / proj chunk
NLOC = 4  # chunks per core
LROWS = NLOC * CHUNK  # local rows per core
STRIPE = 512  # rows per PE stripe
NSTRIP = LROWS // STRIPE
KCH = D // 128

# local chunk offsets and the block pattern (see module docstring)
LOCAL_OFFS = (0, 1, 2, 4)
PATTERN = ((0, 0), (0, 1), (0, 2), (0, 3), (1, 3), (1, 0), (3, 1), (3, 2))

_state = {}

# Set by run for test harnesses that want profile info (see test.py).
LAST_RESULTS = None


def _build():
    from concourse import bacc, tile, mybir

    f32 = mybir.dt.float32
    nc = bacc.Bacc(
        "TRN2", target_bir_lowering=False, debug=False, num_devices=NCORES
    )

    xT_d = nc.dram_tensor("xT4", [D, LROWS], f32, kind="ExternalInput")
    w1_d = nc.dram_tensor("W1", [D, H], f32, kind="ExternalInput")
    b1_d = nc.dram_tensor("b1c", [H, 1], f32, kind="ExternalInput")
    w2_d = nc.dram_tensor("w2c", [H, 1], f32, kind="ExternalInput")
    # w2 replicated across 128 columns: matmul(lhsT=w2b, rhs=seqT) puts
    # proj[f] on every partition in one step (the column broadcast)
    w2b_d = nc.dram_tensor("w2b", [H, 128], f32, kind="ExternalInput")
    b2b_d = nc.dram_tensor("b2b", [128, 1], f32, kind="ExternalInput")
    # 8 blocks of [CHUNK, CHUNK], stacked along rows
    out_d = nc.dram_tensor("out", [8 * CHUNK, CHUNK], f32, kind="ExternalOutput")

    with tile.TileContext(nc) as tc:
        with (
            tc.tile_pool(name="const", bufs=1) as cpool,
            tc.tile_pool(name="xkp", bufs=8) as xkpool,
            tc.tile_pool(name="work", bufs=2) as wpool,
            tc.tile_pool(name="big", bufs=10) as bigpool,
            tc.tile_pool(name="psum", bufs=2, space="PSUM") as pspool,
            tc.tile_pool(name="psbc", bufs=2, space="PSUM") as psbc,
        ):
            # ---- constants ----
            w1_sb = cpool.tile([128, KCH, H], f32)
            nc.sync.dma_start(
                w1_sb[:], w1_d.ap().rearrange("(k p) h -> p k h", p=128)
            )
            b1_sb = cpool.tile([H, 1], f32)
            nc.sync.dma_start(b1_sb[:], b1_d.ap())
            w2_sb = cpool.tile([H, 1], f32)
            nc.sync.dma_start(w2_sb[:], w2_d.ap())
            w2b_sb = cpool.tile([H, 128], f32)
            nc.sync.dma_start(w2b_sb[:], w2b_d.ap())
            b2b_sb = cpool.tile([128, 1], f32)
            nc.sync.dma_start(b2b_sb[:], b2b_d.ap())

            # per-partition proj scalars ([128, CHUNK//128] per local chunk)
            projcol = [
                cpool.tile([128, CHUNK // 128], f32, name=f"projcol{i}")
                for i in range(NLOC)
            ]
            bcol = [
                cpool.tile([128, CHUNK], f32, name=f"bcol{i}") for i in range(NLOC)
            ]

            # ---- per chunk: proj stripes, then its broadcast, then every
            # block that just became ready — so output DMAs start as soon
            # as the first chunk's proj exists and overlap later compute.
            COMPUTE_ORDER = (0, 1, 3, 2)
            ready = {loc: i for i, loc in enumerate(COMPUTE_ORDER)}
            emitted = set()

            def emit_block(k):
                # alternate the adds between DVE and ACT so neither queue
                # backs up behind the other chunk-compute work
                p, q = PATTERN[k]
                for g in range(CHUNK // 128):
                    ot = bigpool.tile([128, CHUNK], f32, tag="ot", name="ot")
                    # all block adds on ACT: DVE stays exclusively on the
                    # chunk-compute path so the PE never waits on it
                    nc.scalar.add(ot[:], bcol[q][:], projcol[p][:, g : g + 1])
                    r0 = k * CHUNK + g * 128
                    nc.sync.dma_start(out_d.ap()[r0 : r0 + 128, :], ot[:])

            for loc in COMPUTE_ORDER:
              for half in range(CHUNK // STRIPE):
                s = loc * (CHUNK // STRIPE) + half
                xk = xkpool.tile([128, KCH, STRIPE], f32, tag="xk")
                nc.sync.dma_start(
                    xk[:],
                    xT_d.ap()[:, s * STRIPE : (s + 1) * STRIPE].rearrange(
                        "(k p) j -> p k j", p=128
                    ),
                )
                seqT_ps = pspool.tile([H, STRIPE], f32, tag="seqT")
                for k in range(KCH):
                    nc.tensor.matmul(
                        seqT_ps[:],
                        w1_sb[:, k, :],
                        xk[:, k, :],
                        start=(k == 0),
                        stop=(k == KCH - 1),
                    )
                seqT_sb = wpool.tile([H, STRIPE], f32, tag="seqT_sb")
                # relu(x + b1) as a fused DVE op, keeping ACT free for the
                # block adds (and avoiding activation-table switching)
                nc.vector.tensor_scalar(
                    seqT_sb[:],
                    seqT_ps[:],
                    b1_sb[:],
                    0.0,
                    op0=mybir.AluOpType.add,
                    op1=mybir.AluOpType.max,
                )
                # broadcast proj of this stripe across all 128 partitions in
                # one matmul, folding b2 into the psum->sbuf copy
                bc_ps = psbc.tile([128, STRIPE], f32, tag="bc")
                nc.tensor.matmul(bc_ps[:], w2b_sb[:], seqT_sb[:])
                nc.vector.tensor_scalar_add(
                    bcol[loc][:, half * STRIPE : (half + 1) * STRIPE],
                    bc_ps[:],
                    b2b_sb[:],
                )
                # local 2 never appears as a block row; skip its scalars
                for c in range(STRIPE // 128) if loc != 2 else ():
                    pc_ps = pspool.tile([128, 1], f32, tag="pc")
                    nc.tensor.matmul(
                        pc_ps[:],
                        seqT_sb[:, c * 128 : (c + 1) * 128],
                        w2_sb[:],
                    )
                    col = half * (STRIPE // 128) + c
                    nc.vector.tensor_copy(projcol[loc][:, col : col + 1], pc_ps[:])

              # emit blocks one stage behind compute: everything ready as of
              # the PREVIOUS chunk. This keeps the next chunk's PSUM-freeing
              # DVE ops ahead of the block adds in each engine's queue, so
              # the PE never stalls on block-emission progress.
              stage = ready[loc]
              for k in range(8):
                  p, q = PATTERN[k]
                  if k not in emitted and max(ready[p], ready[q]) <= max(
                      stage - 1, 0 if stage == 0 else -1
                  ):
                      emitted.add(k)
                      emit_block(k)

            # drain whatever is left (the last two stages)
            for k in range(8):
                if k not in emitted:
                    emitted.add(k)
                    emit_block(k)

    nc.compile()
    return nc


def kernel(gathered_sequences, W1, b1, w2, b2):
    global LAST_RESULTS
    from concourse import bass_utils

    if "nc" not in _state:
        _state["nc"] = _build()
    nc = _state["nc"]

    x = np.ascontiguousarray(gathered_sequences, dtype=np.float32)
    xT = np.ascontiguousarray(x.T)  # [D, N]
    W1 = np.ascontiguousarray(W1, dtype=np.float32)
    b1c = np.ascontiguousarray(np.reshape(b1, (H, 1)), dtype=np.float32)
    w2c = np.ascontiguousarray(np.reshape(w2, (H, 1)), dtype=np.float32)
    w2b = np.ascontiguousarray(np.repeat(w2c, 128, axis=1))
    b2b = np.full((128, 1), np.reshape(b2, ()), dtype=np.float32)

    in_maps = []
    for m in range(NCORES):
        locs = [(m + a) % NCORES for a in LOCAL_OFFS]
        xT4 = np.concatenate(
            [xT[:, L * CHUNK : (L + 1) * CHUNK] for L in locs], axis=1
        )
        in_maps.append(
            {
                "xT4": np.ascontiguousarray(xT4),
                "W1": W1,
                "b1c": b1c,
                "w2c": w2c,
                "w2b": w2b,
                "b2b": b2b,
            }
        )

    res = bass_utils.run_bass_kernel_spmd(nc, in_maps, core_ids=list(range(NCORES)))
    LAST_RESULTS = res

    out = np.empty((N, N), dtype=np.float32)
    for m in range(NCORES):
        locs = [(m + a) % NCORES for a in LOCAL_OFFS]
        blocks = res.results[m]["out"]
        for k, (p, q) in enumerate(PATTERN):
            gr, gc = locs[p], locs[q]
            out[gr * CHUNK : (gr + 1) * CHUNK, gc * CHUNK : (gc + 1) * CHUNK] = (
                blocks[k * CHUNK : (k + 1) * CHUNK, :]
            )
    return out
